# revision 50
# baseline (speedup 1.0000x reference)
"""Trainium2 Bass kernel for nn_BoundaryGreenBranch.

Math (reference):
    bf = relu(relu(bi @ W0 + b0) @ W1 + b1)            # (B, NBC, HID) tiny
    a  = bf @ G0w[:HID] + G0b                          # (B, NBC, HID) tiny
    c  = coords @ G0w[HID:]                            # (B, NINT, HID) small
    h1 = relu(a[:,:,None,:] + c[:,None,:,:])           # (B, NBC, NINT, HID) huge
    h2 = relu(h1 @ G1w + G1b)                          # huge
    u  = (h2 @ G2w + G2b).sum(bc) / NBC                # (B, NINT, 1)

Sharding: 8 cores = 4 batches x 2 halves of NBC (64 bc each). Host does the
tiny encoder stages; each core does its 64bc x 4096int x 64hid block fully
on-chip; host sums the two partial u's per batch (the bc all-reduce).

On-core layout (per quad of 4 bc, pairs packed 2-up on 128 partitions):
    h1 tiles [128, 4096] bf16 built by DVE tensor_scalar (4x mode):
        relu(cT_dup + a'_pair) with a' as per-partition scalar.
    G1: 4 concurrent quadrant matmuls (tile_position) since K=M=64,
        filling the 128x128 PE array -> h2pre in PSUM [128, 1024] (2 banks).
    pass2: ACT activation(Relu, bias=G1b) or DVE tensor_scalar, PSUM->SBUF bf16.
    G2: lhsT=[G2w;G2w] [128,1] matmuls accumulate sum-over-bc into PSUM u
        slots (8 chunks -> 2 banks x 4 col-group partitions).
"""

import numpy as np

B, NBC, HID = 4, 128, 64
NINT = 4096
NCORES = 8
NQUAD = 16  # quads of 4 bc per core (64 bc / 4)
NCH = 8  # interior chunks of 512
CHW = 512  # chunk width

_PROG = {}


def _build_program():
    import concourse.bacc as bacc
    import concourse.tile as tile
    from concourse import mybir

    f32 = mybir.dt.float32
    bf16 = mybir.dt.float16
    Relu = mybir.ActivationFunctionType.Relu
    add = mybir.AluOpType.add
    mx = mybir.AluOpType.max

    nc = bacc.Bacc("TRN2")
    d_ct = nc.declare_dram_parameter("ctdup", [128, NINT], bf16, isOutput=False)
    d_ap = nc.declare_dram_parameter("apairs", [128, 32], f32, isOutput=False)
    d_g1w = nc.declare_dram_parameter("g1w", [128, HID], bf16, isOutput=False)
    d_g2w = nc.declare_dram_parameter("g2w", [128, 1], bf16, isOutput=False)
    d_g1b = nc.declare_dram_parameter("g1b2", [128, 1], f32, isOutput=False)
    d_u = nc.declare_dram_parameter("upart", [NCH, CHW], f32, isOutput=True)

    with tile.TileContext(nc) as tc:
        with (
            tc.tile_pool(name="const", bufs=1) as const,
            tc.tile_pool(name="h1", bufs=3) as h1pool,
            tc.tile_pool(name="h2", bufs=16) as h2pool,
            tc.tile_pool(name="ps", bufs=3, space="PSUM") as pspool,
            tc.tile_pool(name="psu", bufs=1, space="PSUM") as upool,
            tc.tile_pool(name="outp", bufs=1) as outpool,
        ):
            sb_ap = const.tile([128, 32], f32)
            nc.sync.dma_start(out=sb_ap[:], in_=d_ap[:])
            sb_g1w = const.tile([128, HID], bf16)
            nc.sync.dma_start(out=sb_g1w[:], in_=d_g1w[:])
            sb_g2w = const.tile([128, 1], bf16)
            nc.sync.dma_start(out=sb_g2w[:], in_=d_g2w[:])
            sb_g1b = const.tile([128, 1], f32)
            nc.sync.dma_start(out=sb_g1b[:], in_=d_g1b[:])

            # warm the ACT Relu table while the big cT DMA runs
            dummy = const.tile([128, 1], f32)
            nc.scalar.activation(out=dummy[:], in_=sb_g1b[:], func=Relu)

            # 8 parallel DMA queues; first 4 cover pass-1's first half
            sb_ct = const.tile([128, NINT], bf16)
            for i in range(2):
                qs = slice(i * NINT // 2, (i + 1) * NINT // 2)
                nc.gpsimd.dma_start(out=sb_ct[:, qs], in_=d_ct[:, qs])

            psu = [
                upool.tile([128, CHW], f32, name=f"u{i}", tag=f"u{i}")
                for i in range(2)
            ]

            def emit_pass1_piece(q, h1a, h1b, piece):
                """One quarter of next-quad pass1: (tile a/b) x (half lo/hi).
                Spread across the current quad so DVE pass-2 ops interleave."""
                t, lo = divmod(piece, 2)
                hs = slice(lo * NINT // 2, (lo + 1) * NINT // 2)
                tile_, col = (h1a, 2 * q) if t == 0 else (h1b, 2 * q + 1)
                nc.vector.tensor_scalar(
                    out=tile_[:, hs], in0=sb_ct[:, hs],
                    scalar1=sb_ap[:, col : col + 1], scalar2=0.0,
                    op0=add, op1=mx,
                )

            def emit_g2_batch(q, cbase, h2s4):
                """8 G2 matmuls for chunks cbase..cbase+3: per tile-half, the
                4 chunks target 4 distinct PE column groups -> concurrent."""
                ub = psu[cbase // 4]
                for half in range(2):
                    sl = slice(half * CHW, (half + 1) * CHW)
                    for k in range(4):
                        j = 32 * k
                        nc.tensor.matmul(
                            ub[j : j + 1, :], sb_g2w[:], h2s4[k][:, sl],
                            start=(q == 0 and half == 0),
                            stop=(q == NQUAD - 1 and half == 1),
                            tile_position=(0, j),
                        )

            h1a_n = h1pool.tile([128, NINT], bf16, name="h1a", tag="h1a")
            h1b_n = h1pool.tile([128, NINT], bf16, name="h1b", tag="h1b")
            for piece in (0, 2, 1, 3):
                emit_pass1_piece(0, h1a_n, h1b_n, piece)
            PIECE_AT = {1: 0, 3: 2, 4: 1, 6: 3}  # chunk -> next-quad p1 piece
            prev_tail = None  # (q, h2s[4:]) of previous quad
            for q in range(NQUAD):
                h1a, h1b = h1a_n, h1b_n
                if q + 1 < NQUAD:
                    h1a_n = h1pool.tile([128, NINT], bf16, name="h1a", tag="h1a")
                    h1b_n = h1pool.tile([128, NINT], bf16, name="h1b", tag="h1b")
                dve_set = (2, 5) if q % 2 == 0 else (2, 5, 7)
                h2s = []
                for c in range(NCH):
                    sl = slice(c * CHW, (c + 1) * CHW)
                    ps = pspool.tile([128, 2 * CHW], f32, tag="h2pre")
                    nc.tensor.matmul(
                        ps[0:64, 0:CHW], sb_g1w[0:64, :], h1a[0:64, sl],
                        start=True, stop=True, tile_position=(0, 0),
                    )
                    nc.tensor.matmul(
                        ps[64:128, 0:CHW], sb_g1w[64:128, :], h1a[64:128, sl],
                        start=True, stop=True, tile_position=(64, 64),
                    )
                    nc.tensor.matmul(
                        ps[64:128, CHW : 2 * CHW], sb_g1w[0:64, :], h1b[0:64, sl],
                        start=True, stop=True, tile_position=(0, 64),
                    )
                    nc.tensor.matmul(
                        ps[0:64, CHW : 2 * CHW], sb_g1w[64:128, :], h1b[64:128, sl],
                        start=True, stop=True, tile_position=(64, 0),
                    )
                    h2 = h2pool.tile([128, 2 * CHW], bf16, tag="h2")
                    if c in dve_set:
                        nc.vector.tensor_scalar(
                            out=h2[:], in0=ps[:],
                            scalar1=sb_g1b[:], scalar2=0.0, op0=add, op1=mx,
                        )
                    else:
                        nc.scalar.activation(
                            out=h2[:], in_=ps[:], func=Relu,
                            bias=sb_g1b[:], scale=1.0,
                        )
                    h2s.append(h2)
                    if q + 1 < NQUAD and c in PIECE_AT:
                        emit_pass1_piece(q + 1, h1a_n, h1b_n, PIECE_AT[c])
                    if c == 1 and prev_tail is not None:
                        emit_g2_batch(prev_tail[0], 4, prev_tail[1])
                emit_g2_batch(q, 0, h2s[0:4])
                prev_tail = (q, h2s[4:8])

            def evac_u(i):
                so = outpool.tile([128, CHW], f32, name=f"so{i}", tag=f"so{i}")
                if i == 0:
                    nc.vector.tensor_copy(out=so[:], in_=psu[i][:])
                else:
                    nc.scalar.copy(out=so[:], in_=psu[i][:])
                for r in range(4):
                    nc.sync.dma_start(
                        out=d_u[4 * i + r : 4 * i + r + 1, :],
                        in_=so[32 * r : 32 * r + 1, :],
                    )

            evac_u(0)  # U0 is done; overlaps the final U1 batch
            emit_g2_batch(NQUAD - 1, 4, prev_tail[1])
            evac_u(1)

    nc.compile()
    return nc


def _relu(x):
    return np.maximum(x, 0.0)


def _prepare_in_maps(
    boundary_info, interior_coords, W0, b0, W1, b1,
    G0w, G0b, G1w, G1b, G2w, G2b,
):
    import ml_dtypes

    bf16 = np.float16
    bi = np.asarray(boundary_info, np.float32)
    coords = np.asarray(interior_coords, np.float32)
    W0, b0, W1, b1 = (np.asarray(t, np.float32) for t in (W0, b0, W1, b1))
    G0w, G0b, G1w, G1b, G2w, G2b = (
        np.asarray(t, np.float32) for t in (G0w, G0b, G1w, G1b, G2w, G2b)
    )

    # tiny encoder stages on host
    bf = _relu(bi @ W0 + b0)
    bf = _relu(bf @ W1 + b1)
    a = bf @ G0w[:HID] + G0b  # (B, NBC, HID)
    cint = coords @ G0w[HID:]  # (B, NINT, HID)

    g1w_sb = np.vstack([G1w, G1w]).astype(bf16)
    g2w_sb = np.vstack([G2w, G2w]).astype(bf16)
    g1b2 = np.concatenate([G1b, G1b]).reshape(128, 1).astype(np.float32)

    in_maps = []
    for core in range(NCORES):
        b, half = divmod(core, 2)
        cT = np.ascontiguousarray(cint[b].T)  # (64, 4096)
        ctdup = np.vstack([cT, cT]).astype(bf16)
        asl = a[b, half * 64 : (half + 1) * 64]  # (64 bc, 64 hid)
        apairs = np.ascontiguousarray(asl.reshape(32, 128).T).astype(np.float32)
        in_maps.append(
            {
                "ctdup": ctdup,
                "apairs": apairs,
                "g1w": g1w_sb,
                "g2w": g2w_sb,
                "g1b2": g1b2,
            }
        )
    return in_maps


def _run(in_maps, **kwargs):
    from concourse.bass_utils import run_bass_kernel_spmd

    if "nc" not in _PROG:
        _PROG["nc"] = _build_program()
    return run_bass_kernel_spmd(_PROG["nc"], in_maps, list(range(NCORES)), **kwargs)


def kernel(
    boundary_info, interior_coords, W0, b0, W1, b1,
    G0w, G0b, G1w, G1b, G2w, G2b, interior_h, interior_w,
):
    in_maps = _prepare_in_maps(
        boundary_info, interior_coords, W0, b0, W1, b1,
        G0w, G0b, G1w, G1b, G2w, G2b,
    )
    res = _run(in_maps)

    u = np.zeros((B, NINT), np.float64)
    for core in range(NCORES):
        b = core // 2
        u[b] += res.results[core]["upart"].reshape(NINT).astype(np.float64)
    u = (u / NBC + np.asarray(G2b, np.float32)[0]).astype(np.float32)
    return u.reshape(B, 1, int(interior_h), int(interior_w))
